# revision 29
# baseline (speedup 1.0000x reference)
"""DGCNConv (GNN message passing) Trainium2 kernel, 8-core SPMD.

Strategy (graph/data parallel, per sharding hint):
- Nodes are partitioned into 8 contiguous ranges of 6250. Core c owns the
  dst-range edges for agg_in and the src-range edges for agg_out.
- Per direction, edges are sorted (gather-table half, exact target node):
  each direction is two giant contiguous gather streams; SWDGE dma_gather
  moves 128 B per edge (fp16 row gathered out of a 256 B-strided table via
  a narrow-gather emitter; the ISA only constrains the row STRIDE to 256 B).
- Segment reduction: edges stream through 128-position chunks; each chunk
  is matmul-scattered into per-64-node-window PSUM accumulators using
  on-chip one-hot masks (is_equal vs iota over 64 columns). Windows are not
  position-padded: chunks may span window boundaries, in which case the
  chunk gets one mask per overlapped window (values outside [0,64) simply
  match nothing). This removes the per-segment pad-to-128 waste and halves
  the mask bandwidth vs 128-wide blocks (DVE was a main bottleneck).
- Per 128-node block (2 windows, interleaved with the gather stream):
  accumulate both directions, copy window sums out of PSUM, then run the
  epilogue outT = W_self@xT + W_in@agg_inT + W_out@agg_outT, ReLU (+BN
  partial sums). Gathers are prefetched LA blocks ahead.
- Tail: cross-core AllReduce of BN partials, fused scale/shift normalize
  (into the retired xT buffer), writeback.

The per-(half,window) budgets are the max over cores so all 8 cores run one
identical program (SPMD NEFF) on per-core data; per-core shortfall positions
carry dcmp=-10000 (mask 0) and idx=0.
"""

import sys

if "/opt/trn_rl_repo" not in sys.path:
    sys.path.insert(0, "/opt/trn_rl_repo")

import numpy as np

N_NODES = 50000
N_EDGES = 800000
D = 64
N_CORES = 8
NPC = N_NODES // N_CORES          # 6250 nodes per core
NBLK = (NPC + 127) // 128         # 49 epilogue blocks per core
W = 64                            # scatter window width (mask columns)
NWIN = (NPC + W - 1) // W         # 98 windows per core
HALF = N_NODES // 2               # 25000, int16-safe gather base split
BN_EPS = 1e-5
GCAP = 1024                       # positions per dma_gather instruction
# single_packet=True packs one instruction's per-engine descriptors into ONE
# SDMA packet, capped at 64 descs/engine -> max 16*64 = 1024 positions.
SP = True                         # single_packet for gather instructions
LA = 12                           # gather prefetch lookahead, in node blocks
SCRATCH = 16384                   # SWDGE descriptor ring bytes (16B/desc)
MB = 32                           # mask instances per one-hot build batch
PAD = -10000.0                    # dcmp value for padding positions
ABLATE = set()                    # timing experiments: {"gather"} | {"masks"}


# ---------------------------------------------------------------- host prep

def _route_direction(t_all, g_all):
    """Route edges (t = reduce-target node id, g = gather node id) to cores.

    Returns (layout, per_core): layout is the static position/instance map
    shared by all cores; per_core holds each core's idx/dcmp tables.
    """
    core_of = t_all // NPC
    per_core_edges = []
    for c in range(N_CORES):
        m = core_of == c
        t = t_all[m] - c * NPC
        g = g_all[m]
        half = (g >= HALF).astype(np.int64)
        order = np.lexsort((g, t, half))
        per_core_edges.append((t[order], g[order], half[order]))

    # static budgets per (half, window): max over cores (no 128 round-up)
    budgets = np.zeros((2, NWIN), np.int64)
    for c in range(N_CORES):
        t, g, half = per_core_edges[c]
        w = t // W
        cnt = np.bincount(half * NWIN + w, minlength=2 * NWIN).reshape(2, NWIN)
        budgets = np.maximum(budgets, cnt)
    # each half stream must be whole chunks (chunks cannot span halves)
    for h in range(2):
        budgets[h, NWIN - 1] += (-budgets[h].sum()) % 128

    # positions: half-major, window order within half
    seg_start = np.zeros((2, NWIN), np.int64)
    pos = 0
    half_start = [0, 0]
    half_len = [0, 0]
    for h in range(2):
        half_start[h] = pos
        for w in range(NWIN):
            seg_start[h, w] = pos
            pos += budgets[h, w]
        half_len[h] = pos - half_start[h]
    total = pos
    nch = total // 128

    # gather runs: each half stream split at GCAP
    runs = []  # (pos0, npos, half)
    for h in range(2):
        p0 = half_start[h]
        pend = half_start[h] + half_len[h]
        while p0 < pend:
            take = min(pend - p0, GCAP)
            runs.append((p0, int(take), h))
            p0 += take

    chunk_run = np.zeros(nch, np.int64)
    run_c0 = np.zeros(len(runs), np.int64)
    for ri, (p0, n, h) in enumerate(runs):
        run_c0[ri] = p0 // 128
        chunk_run[p0 // 128:(p0 + n) // 128] = ri

    # mask instances, in exact kernel consumption order:
    #   for block b: for w in (2b, 2b+1): for h in (0,1): chunks overlapping
    #   the (h, w) segment. Each instance is one (chunk, window) matmul.
    insts = []                     # (w, h, chunk)
    block_insts = []               # per block: [(w, [(inst_idx, chunk), ...])]
    for b in range(NBLK):
        wlist = []
        for w in (2 * b, 2 * b + 1):
            if w >= NWIN:
                continue
            ilist = []
            for h in range(2):
                if budgets[h, w] == 0:
                    continue
                c0 = int(seg_start[h, w]) // 128
                c1 = -(-int(seg_start[h, w] + budgets[h, w]) // 128)
                for c in range(c0, c1):
                    ilist.append((len(insts), c))
                    insts.append((w, h, c))
            wlist.append((w, ilist))
        block_insts.append(wlist)
    n_inst = len(insts)

    # chunk -> first block (for gather prefetch lookahead)
    pos_w = np.zeros(total, np.int64)
    for h in range(2):
        for w in range(NWIN):
            s = int(seg_start[h, w])
            pos_w[s:s + int(budgets[h, w])] = w
    run_first_blk = [int(pos_w[p0]) // 2 for (p0, n, h) in runs]

    # per-core position arrays
    per_core = []
    for c in range(N_CORES):
        t, g, half = per_core_edges[c]
        idx = np.zeros(total, np.int16)      # gather idx rel to half base
        tval = np.full(total, PAD, np.float32)
        w = t // W
        key = half * NWIN + w
        cnt = np.bincount(key, minlength=2 * NWIN)
        estart = np.zeros(2 * NWIN, np.int64)
        estart[1:] = np.cumsum(cnt)[:-1]
        for h in range(2):
            for wi in range(NWIN):
                n = int(cnt[h * NWIN + wi])
                if n == 0:
                    continue
                e0 = int(estart[h * NWIN + wi])
                p0 = int(seg_start[h, wi])
                idx[p0:p0 + n] = (g[e0:e0 + n] - h * HALF).astype(np.int16)
                tval[p0:p0 + n] = t[e0:e0 + n]
        idx_wrapped = np.tile(
            np.ascontiguousarray(idx.reshape(-1, 16).T), (8, 1))
        # dcmp: one column per instance, value = t - W*w (or PAD)
        dcmp = np.empty((128, n_inst), np.float16)
        for ii, (wi, h, ci) in enumerate(insts):
            col = tval[128 * ci:128 * (ci + 1)] - W * wi
            col[tval[128 * ci:128 * (ci + 1)] == PAD] = PAD
            dcmp[:, ii] = col.astype(np.float16)
        per_core.append((idx_wrapped, np.ascontiguousarray(dcmp)))

    layout = dict(total=total, nch=nch, runs=runs, chunk_run=chunk_run,
                  run_c0=run_c0, run_first_blk=run_first_blk,
                  block_insts=block_insts, n_inst=n_inst)
    return layout, per_core


# ---------------------------------------------------------------- program

def _dma_gather_narrow(nc, out_ap, in_ap, idxs_ap, num_idxs, elem_size,
                       elem_step, queue_num, single_packet=True):
    """dma_gather with elem_size_bytes < 256 (non-transpose only).

    bass.dma_gather asserts elem_size_bytes % 256 == 0, but the Q7 ucode
    only needs that for transpose=True; the non-transpose path supports any
    elem_size (one packet per index below 16 KiB). What the ISA does require
    is the table ROW STRIDE in multiples of 256 B (stride_bytes_256 field),
    which the caller satisfies via elem_step. Emitting the instruction here
    keeps the x table 256 B-strided while moving only 128 B per edge.
    """
    import concourse.mybir as mybir

    g = nc.gpsimd
    assert idxs_ap.dtype == mybir.dt.int16
    assert in_ap.dtype == out_ap.dtype
    dt_sz = mybir.dt.size(in_ap.dtype)
    assert in_ap.ap[0][0] == elem_step
    assert in_ap.ap[-1][1] == out_ap.ap[-1][1] == elem_size
    assert num_idxs % 128 == 0
    assert out_ap.ap[0][1] * out_ap.ap[1][1] == num_idxs
    stride_bytes = elem_step * dt_sz
    assert stride_bytes % 256 == 0 and stride_bytes // 256 < 256
    _in_ap = g.lower_ap_dma(in_ap, for_custom_bir_dma=True)
    _idxs_ap = g.lower_ap(idxs_ap)
    _out_ap = g.lower_ap(out_ap)
    return g.add_instruction(
        mybir.InstDMAGatherAnt(
            name=nc.get_next_instruction_name(),
            ins=[*_in_ap, _idxs_ap, g.lower_val_access(g.to_reg(num_idxs))],
            outs=[_out_ap],
            transpose=False,
            num_idxs=num_idxs,
            elem_size=elem_size,
            stride_bytes_256=stride_bytes // 256,
            gen_mode=0,
            single_packet=single_packet,
            queue_num=queue_num,
            sbuf_tokens_per_rank=0,
            sbuf_free_dim_per_rank=0,
            sbuf_free_dim_pad_per_rank=0,
            sbuf_byte_offset=0,
        )
    )


def _build_program(lay_in, lay_out):
    import concourse.bacc as bacc
    import concourse.mybir as mybir
    from concourse import tile
    from concourse import library_config

    f32, f16, i16 = mybir.dt.float32, mybir.dt.float16, mybir.dt.int16
    nc = bacc.Bacc(None, target_bir_lowering=False, debug=False,
                   dynamic_dma_scratch_size=SCRATCH, num_swdge_queues=4)

    xdup = nc.dram_tensor("xdup", [N_NODES, 2 * D], f16, kind="ExternalInput")
    xT_d = nc.dram_tensor("xT", [D, NPC], f32, kind="ExternalInput")
    Wt_d = nc.dram_tensor("Wt", [D, 3 * D], f32, kind="ExternalInput")
    gb_d = nc.dram_tensor("gb", [D, 2], f32, kind="ExternalInput")
    out_d = nc.dram_tensor("out", [D, NPC], f32, kind="ExternalOutput")
    cc_in = nc.dram_tensor("cc_in", [D, 2], f32)
    cc_out = nc.dram_tensor("cc_out", [D, 2], f32, addr_space="Shared")

    lays = {"in": lay_in, "out": lay_out}
    idx_d, dcmp_d = {}, {}
    for dk in ("in", "out"):
        idx_d[dk] = nc.dram_tensor(
            f"idx_{dk}", [128, lays[dk]["total"] // 16], i16,
            kind="ExternalInput")
        dcmp_d[dk] = nc.dram_tensor(
            f"dcmp_{dk}", [128, lays[dk]["n_inst"]], f16,
            kind="ExternalInput")

    with tile.TileContext(nc) as tc:
        nc.gpsimd.load_library(library_config.mlp)
        with (
            tc.tile_pool(name="const", bufs=1) as cpool,
            tc.tile_pool(name="gath", bufs=28) as gpool,
            tc.tile_pool(name="mb", bufs=8) as mpool,
            tc.tile_pool(name="dr", bufs=4) as dpool,
            tc.tile_pool(name="aggt", bufs=6) as apool,
            tc.tile_pool(name="agg_ps", bufs=5, space="PSUM") as agg_pspool,
            tc.tile_pool(name="out_ps", bufs=3, space="PSUM") as out_pspool,
        ):
            # --- constants
            xT = cpool.tile([D, NPC], f32, tag="xT")
            nc.sync.dma_start(xT[:], xT_d[:])
            Wt = cpool.tile([D, 3 * D], f32, tag="Wt")
            nc.sync.dma_start(Wt[:], Wt_d[:])
            gb = cpool.tile([D, 2], f32, tag="gb")
            nc.sync.dma_start(gb[:], gb_d[:])
            iota_i = cpool.tile([128, MB, W], i16, tag="iota_i")
            nc.gpsimd.iota(iota_i[:], [[0, MB], [1, W]], base=0,
                           channel_multiplier=0)
            iota_f = cpool.tile([128, MB, W], f16, tag="iota_f")
            nc.vector.tensor_copy(iota_f[:], iota_i[:])

            idx_t, dcmp_t = {}, {}
            for dk in ("in", "out"):
                idx_t[dk] = cpool.tile(
                    [128, lays[dk]["total"] // 16], i16, tag=f"idx{dk}",
                    name=f"idx_t_{dk}")
                nc.sync.dma_start(idx_t[dk][:], idx_d[dk][:])
                dcmp_t[dk] = cpool.tile(
                    [128, lays[dk]["n_inst"]], f16, tag=f"dc{dk}",
                    name=f"dcmp_t_{dk}")
                nc.sync.dma_start(dcmp_t[dk][:], dcmp_d[dk][:])

            g_tiles = {"in": {}, "out": {}}
            m_tiles = {"in": {}, "out": {}}
            _bcast_failed = []

            g_dummy = None
            if "gather" in ABLATE or "detach" in ABLATE:
                g_dummy = cpool.tile([128, GCAP // 128, D], f16, tag="gdum")
                nc.vector.memset(g_dummy[:], 0.0)

            def emit_run(dk, ri):
                lay = lays[dk]
                p0, npos, h = lay["runs"][ri]
                gt = gpool.tile([128, npos // 128, D], f16, tag="g",
                                name=f"g_{dk}_{ri}")
                src = xdup[h * HALF:(h + 1) * HALF, 0:D]
                qn = (0 if dk == "in" else 2) + h  # one queue per stream
                _dma_gather_narrow(
                    nc, gt[:], src, idx_t[dk][:, p0 // 16:(p0 + npos) // 16],
                    npos, D, 2 * D, queue_num=qn, single_packet=SP)
                g_tiles[dk][ri] = gt

            def emit_batch(dk, k):
                """Build the one-hot masks for instance batch k."""
                nb = min(MB, lays[dk]["n_inst"] - k * MB)
                bcast = dcmp_t[dk][:, k * MB:k * MB + nb].unsqueeze(2) \
                    .broadcast_to([128, nb, W])
                mt = mpool.tile([128, nb, W], f16, tag="m",
                                name=f"m_{dk}_{k}")
                try:
                    nc.vector.tensor_tensor(
                        mt[:], iota_f[:, :nb, :], bcast,
                        op=mybir.AluOpType.is_equal)
                except Exception:
                    _bcast_failed.append(k)
                    dr = dpool.tile([128, nb, W], f16, tag="drep",
                                    name=f"dr_{dk}_{k}")
                    nc.vector.tensor_copy(dr[:], bcast)
                    nc.vector.tensor_tensor(
                        mt[:], iota_f[:, :nb, :], dr[:],
                        op=mybir.AluOpType.is_equal)
                m_tiles[dk][k] = mt

            # --- per-block: accumulate windows, then epilogue
            r_sb = cpool.tile([D, NPC], f32, tag="r")
            sums = cpool.tile([D, NBLK], f32, tag="sums")
            sumsq = cpool.tile([D, NBLK], f32, tag="sumsq")
            sq_scr = cpool.tile([D, 128], f32, tag="sq")

            prefetch = sorted(
                [(lays[dk]["run_first_blk"][ri], dk, ri)
                 for dk in ("in", "out")
                 for ri in range(len(lays[dk]["runs"]))])
            pf_ptr = [0]

            def prefetch_to(blk):
                while (pf_ptr[0] < len(prefetch)
                       and prefetch[pf_ptr[0]][0] <= blk):
                    _, dk, ri = prefetch[pf_ptr[0]]
                    if ri not in g_tiles[dk] and "gather" not in ABLATE:
                        emit_run(dk, ri)
                    pf_ptr[0] += 1

            for b in range(NBLK):
                prefetch_to(b + LA)
                agg_t = {}
                for dk in ("in", "out"):
                    lay = lays[dk]
                    at = apool.tile([D, 128], f32, tag="aggt",
                                    name=f"at_{dk}_{b}")
                    covered = 0
                    for (w, ilist) in lay["block_insts"][b]:
                        col0 = (w % 2) * W
                        if not ilist or "masks" in ABLATE:
                            if "masks" in ABLATE and ilist \
                               and "gather" not in ABLATE:
                                for (ii, c) in ilist:
                                    ri = int(lay["chunk_run"][c])
                                    if ri not in g_tiles[dk]:
                                        emit_run(dk, ri)
                            nc.vector.memset(at[:, col0:col0 + W], 0.0)
                            covered += 1
                            continue
                        aps = agg_pspool.tile([D, W], f32, tag="aggps",
                                              name=f"aps_{dk}_{b}_{w}")
                        for j, (ii, c) in enumerate(ilist):
                            k, r = divmod(ii, MB)
                            if k not in m_tiles[dk]:
                                emit_batch(dk, k)
                            ri = int(lay["chunk_run"][c])
                            if "gather" in ABLATE:
                                gt = g_dummy
                            else:
                                if ri not in g_tiles[dk]:
                                    emit_run(dk, ri)
                                gt = g_tiles[dk][ri]
                            if "detach" in ABLATE:
                                gt = g_dummy
                            nc.tensor.matmul(
                                aps[:],
                                gt[:, (c - int(lay["run_c0"][ri]))
                                   % (GCAP // 128), 0:D],
                                m_tiles[dk][k][:, r, :],
                                start=(j == 0), stop=(j == len(ilist) - 1))
                        nc.scalar.activation(at[:, col0:col0 + W], aps[:],
                                             mybir.ActivationFunctionType.Copy)
                        covered += 1
                    if covered < 2:  # trailing window beyond NPC
                        nc.vector.memset(at[:, W:128], 0.0)
                    agg_t[dk] = at

                ncols = min(128, NPC - b * 128)
                ops = out_pspool.tile([D, ncols], f32, tag="outps",
                                      name=f"ops_{b}")
                nc.tensor.matmul(ops[:], Wt[:, 0:D],
                                 xT[:, b * 128:b * 128 + ncols],
                                 start=True, stop=False)
                nc.tensor.matmul(ops[:], Wt[:, D:2 * D],
                                 agg_t["in"][:, 0:ncols],
                                 start=False, stop=False)
                nc.tensor.matmul(ops[:], Wt[:, 2 * D:3 * D],
                                 agg_t["out"][:, 0:ncols],
                                 start=False, stop=True)
                nc.scalar.activation(
                    r_sb[:, b * 128:b * 128 + ncols], ops[:],
                    mybir.ActivationFunctionType.Relu,
                    accum_out=sums[:, b:b + 1])
                nc.scalar.activation(
                    sq_scr[:, 0:ncols], r_sb[:, b * 128:b * 128 + ncols],
                    mybir.ActivationFunctionType.Square,
                    accum_out=sumsq[:, b:b + 1])

            # --- BN stats allreduce
            part = cpool.tile([D, 2], f32, tag="part")
            nc.vector.tensor_reduce(part[:, 0:1], sums[:],
                                    mybir.AxisListType.X, mybir.AluOpType.add)
            nc.vector.tensor_reduce(part[:, 1:2], sumsq[:],
                                    mybir.AxisListType.X, mybir.AluOpType.add)
            nc.sync.dma_start(cc_in[:], part[:])
            nc.gpsimd.collective_compute(
                "AllReduce", mybir.AluOpType.add,
                replica_groups=[list(range(N_CORES))],
                ins=[cc_in[:]], outs=[cc_out[:]])
            tot = cpool.tile([D, 2], f32, tag="tot")
            nc.sync.dma_start(tot[:], cc_out[:])

            # --- scale/shift
            stats = cpool.tile([D, 8], f32, tag="stats")
            mean, ex2 = stats[:, 0:1], stats[:, 1:2]
            var, std = stats[:, 2:3], stats[:, 3:4]
            inv, scale = stats[:, 4:5], stats[:, 5:6]
            shift, tmp = stats[:, 6:7], stats[:, 7:8]
            inv_n = 1.0 / float(N_NODES)
            nc.vector.tensor_scalar_mul(mean, tot[:, 0:1], inv_n)
            nc.vector.tensor_scalar_mul(ex2, tot[:, 1:2], inv_n)
            nc.vector.tensor_tensor(tmp, mean, mean, op=mybir.AluOpType.mult)
            nc.vector.tensor_tensor(var, ex2, tmp,
                                    op=mybir.AluOpType.subtract)
            nc.vector.tensor_scalar_add(var, var, BN_EPS)
            nc.scalar.activation(std, var, mybir.ActivationFunctionType.Sqrt)
            nc.vector.reciprocal(inv, std)
            nc.vector.tensor_tensor(scale, gb[:, 0:1], inv,
                                    op=mybir.AluOpType.mult)
            nc.vector.tensor_tensor(tmp, mean, scale,
                                    op=mybir.AluOpType.mult)
            nc.vector.tensor_tensor(shift, gb[:, 1:2], tmp,
                                    op=mybir.AluOpType.subtract)

            # --- normalize into the retired xT buffer + writeback
            nc.vector.tensor_scalar(xT[:], r_sb[:], scale, shift,
                                    op0=mybir.AluOpType.mult,
                                    op1=mybir.AluOpType.add)
            nc.sync.dma_start(out_d[:], xT[:])

    if _bcast_failed:
        print(f"note: mask broadcast tensor_tensor fell back to copy "
              f"for {len(_bcast_failed)} batches", file=sys.stderr)
    nc.finalize()
    return nc


# ---------------------------------------------------------------- kernel

def prepare(x, edge_index, num_nodes=None, W_in=None, W_out=None,
            W_self=None, gamma=None, beta=None):
    """Build the bass program and per-core input maps."""
    x = np.asarray(x, np.float32)
    edge_index = np.asarray(edge_index, np.int64)
    W_in = np.asarray(W_in, np.float32)
    W_out = np.asarray(W_out, np.float32)
    W_self = np.asarray(W_self, np.float32)
    gamma = np.asarray(gamma, np.float32)
    beta = np.asarray(beta, np.float32)
    assert x.shape == (N_NODES, D) and edge_index.shape == (2, N_EDGES)

    src, dst = edge_index[0], edge_index[1]
    lay_in, pc_in = _route_direction(dst, src)    # agg_in: reduce by dst
    lay_out, pc_out = _route_direction(src, dst)  # agg_out: reduce by src

    nc = _build_program(lay_in, lay_out)

    x16 = x.astype(np.float16)
    xdup = np.concatenate([x16, x16], axis=1)
    Wt = np.concatenate([W_self.T, W_in.T, W_out.T], axis=1).astype(np.float32)
    Wt = np.ascontiguousarray(Wt)
    gb = np.ascontiguousarray(np.stack([gamma, beta], axis=1).astype(np.float32))

    in_maps = []
    for c in range(N_CORES):
        xT_c = np.ascontiguousarray(x[c * NPC:(c + 1) * NPC].T)
        in_maps.append({
            "xdup": xdup,
            "xT": xT_c,
            "Wt": Wt,
            "gb": gb,
            "idx_in": pc_in[c][0], "dcmp_in": pc_in[c][1],
            "idx_out": pc_out[c][0], "dcmp_out": pc_out[c][1],
        })
    return nc, in_maps


def postprocess(results):
    outT = np.concatenate([r["out"] for r in results], axis=1)
    return np.ascontiguousarray(outT.T).astype(np.float32)


def kernel(x, edge_index, num_nodes=None, W_in=None, W_out=None,
           W_self=None, gamma=None, beta=None):
    from concourse.bass_utils import run_bass_kernel_spmd

    nc, in_maps = prepare(x, edge_index, num_nodes, W_in, W_out,
                          W_self, gamma, beta)
    res = run_bass_kernel_spmd(nc, in_maps, core_ids=list(range(N_CORES)))
    return postprocess(res.results)


# revision 33
# speedup vs baseline: 1.1273x; 1.1273x over previous
"""DGCNConv (GNN message passing) Trainium2 kernel, 8-core SPMD.

Strategy (graph/data parallel, per sharding hint):
- Nodes are partitioned into 8 contiguous ranges of 6250. Core c owns the
  dst-range edges for agg_in and the src-range edges for agg_out.
- Per direction, edges are sorted (gather-table half, exact target node):
  each direction is two giant contiguous gather streams; SWDGE dma_gather
  moves 128 B per edge (fp16 row gathered out of a 256 B-strided table via
  a narrow-gather emitter; the ISA only constrains the row STRIDE to 256 B).
- Segment reduction: edges stream through 128-position chunks; each chunk
  is matmul-scattered into per-64-node-window PSUM accumulators using
  on-chip one-hot masks (is_equal vs iota over 64 columns). Windows are not
  position-padded: chunks may span window boundaries, in which case the
  chunk gets one mask per overlapped window (values outside [0,64) simply
  match nothing). This removes the per-segment pad-to-128 waste and halves
  the mask bandwidth vs 128-wide blocks (DVE was a main bottleneck).
- Per 128-node block (2 windows, interleaved with the gather stream):
  accumulate both directions, copy window sums out of PSUM, then run the
  epilogue outT = W_self@xT + W_in@agg_inT + W_out@agg_outT, ReLU (+BN
  partial sums). Gathers are prefetched LA blocks ahead.
- Tail: cross-core AllReduce of BN partials, fused scale/shift normalize
  (into the retired xT buffer), writeback.

The per-(half,window) budgets are the max over cores so all 8 cores run one
identical program (SPMD NEFF) on per-core data; per-core shortfall positions
carry dcmp=-10000 (mask 0) and idx=0.
"""

import sys

if "/opt/trn_rl_repo" not in sys.path:
    sys.path.insert(0, "/opt/trn_rl_repo")

import numpy as np

N_NODES = 50000
N_EDGES = 800000
D = 64
N_CORES = 8
NPC = N_NODES // N_CORES          # 6250 nodes per core
NBLK = (NPC + 127) // 128         # 49 epilogue blocks per core
W = 64                            # scatter window width (mask columns)
NWIN = (NPC + W - 1) // W         # 98 windows per core
HALF = N_NODES // 2               # 25000, int16-safe gather base split
BN_EPS = 1e-5
GCAP = 1024                       # positions per dma_gather instruction
# single_packet=True packs one instruction's per-engine descriptors into ONE
# SDMA packet, capped at 64 descs/engine -> max 16*64 = 1024 positions.
SP = True                         # single_packet for gather instructions
LA = 12                           # gather prefetch lookahead, in node blocks
SCRATCH = 16384                   # SWDGE descriptor ring bytes (16B/desc)
MB = 32                           # mask instances per one-hot build batch
PAD = -10000.0                    # dcmp value for padding positions
ABLATE = set()                    # timing experiments: {"gather"} | {"masks"}


# ---------------------------------------------------------------- host prep

def _route_direction(t_all, g_all):
    """Route edges (t = reduce-target node id, g = gather node id) to cores.

    Returns (layout, per_core): layout is the static position/instance map
    shared by all cores; per_core holds each core's idx/dcmp tables.
    """
    core_of = t_all // NPC
    per_core_edges = []
    for c in range(N_CORES):
        m = core_of == c
        t = t_all[m] - c * NPC
        g = g_all[m]
        half = (g >= HALF).astype(np.int64)
        # group by (half, window); ascending g within a segment (the order
        # inside a segment is free - dcmp carries exact targets - and sorted
        # gather addresses are kinder to HBM)
        order = np.lexsort((g, t // W, half))
        per_core_edges.append((t[order], g[order], half[order]))

    # static budgets per (half, window): max over cores (no 128 round-up)
    budgets = np.zeros((2, NWIN), np.int64)
    for c in range(N_CORES):
        t, g, half = per_core_edges[c]
        w = t // W
        cnt = np.bincount(half * NWIN + w, minlength=2 * NWIN).reshape(2, NWIN)
        budgets = np.maximum(budgets, cnt)
    # each half stream must be whole chunks (chunks cannot span halves)
    for h in range(2):
        budgets[h, NWIN - 1] += (-budgets[h].sum()) % 128

    # positions: half-major, window order within half
    seg_start = np.zeros((2, NWIN), np.int64)
    pos = 0
    half_start = [0, 0]
    half_len = [0, 0]
    for h in range(2):
        half_start[h] = pos
        for w in range(NWIN):
            seg_start[h, w] = pos
            pos += budgets[h, w]
        half_len[h] = pos - half_start[h]
    total = pos
    nch = total // 128

    # gather runs: each half stream split at GCAP
    runs = []  # (pos0, npos, half)
    for h in range(2):
        p0 = half_start[h]
        pend = half_start[h] + half_len[h]
        while p0 < pend:
            take = min(pend - p0, GCAP)
            runs.append((p0, int(take), h))
            p0 += take

    chunk_run = np.zeros(nch, np.int64)
    run_c0 = np.zeros(len(runs), np.int64)
    for ri, (p0, n, h) in enumerate(runs):
        run_c0[ri] = p0 // 128
        chunk_run[p0 // 128:(p0 + n) // 128] = ri

    # mask instances, in exact kernel consumption order:
    #   for block b: for w in (2b, 2b+1): for h in (0,1): chunks overlapping
    #   the (h, w) segment. Each instance is one (chunk, window) matmul.
    insts = []                     # (w, h, chunk)
    block_insts = []               # per block: [(w, [(inst_idx, chunk), ...])]
    for b in range(NBLK):
        wlist = []
        for w in (2 * b, 2 * b + 1):
            if w >= NWIN:
                continue
            ilist = []
            for h in range(2):
                if budgets[h, w] == 0:
                    continue
                c0 = int(seg_start[h, w]) // 128
                c1 = -(-int(seg_start[h, w] + budgets[h, w]) // 128)
                for c in range(c0, c1):
                    ilist.append((len(insts), c))
                    insts.append((w, h, c))
            wlist.append((w, ilist))
        block_insts.append(wlist)
    n_inst = len(insts)

    # chunk -> first block (for gather prefetch lookahead)
    pos_w = np.zeros(total, np.int64)
    for h in range(2):
        for w in range(NWIN):
            s = int(seg_start[h, w])
            pos_w[s:s + int(budgets[h, w])] = w
    run_first_blk = [int(pos_w[p0]) // 2 for (p0, n, h) in runs]

    # per-core position arrays
    per_core = []
    for c in range(N_CORES):
        t, g, half = per_core_edges[c]
        idx = np.zeros(total, np.int16)      # gather idx rel to half base
        tval = np.full(total, PAD, np.float32)
        w = t // W
        key = half * NWIN + w
        cnt = np.bincount(key, minlength=2 * NWIN)
        estart = np.zeros(2 * NWIN, np.int64)
        estart[1:] = np.cumsum(cnt)[:-1]
        for h in range(2):
            for wi in range(NWIN):
                n = int(cnt[h * NWIN + wi])
                if n == 0:
                    continue
                e0 = int(estart[h * NWIN + wi])
                p0 = int(seg_start[h, wi])
                idx[p0:p0 + n] = (g[e0:e0 + n] - h * HALF).astype(np.int16)
                tval[p0:p0 + n] = t[e0:e0 + n]
        idx_wrapped = np.tile(
            np.ascontiguousarray(idx.reshape(-1, 16).T), (8, 1))
        # dcmp: one column per instance, value = t - W*w (or PAD)
        dcmp = np.empty((128, n_inst), np.float16)
        for ii, (wi, h, ci) in enumerate(insts):
            col = tval[128 * ci:128 * (ci + 1)] - W * wi
            col[tval[128 * ci:128 * (ci + 1)] == PAD] = PAD
            dcmp[:, ii] = col.astype(np.float16)
        per_core.append((idx_wrapped, np.ascontiguousarray(dcmp)))

    layout = dict(total=total, nch=nch, runs=runs, chunk_run=chunk_run,
                  run_c0=run_c0, run_first_blk=run_first_blk,
                  block_insts=block_insts, n_inst=n_inst)
    return layout, per_core


# ---------------------------------------------------------------- program

def _dma_gather_narrow(nc, out_ap, in_ap, idxs_ap, num_idxs, elem_size,
                       elem_step, queue_num, single_packet=True):
    """dma_gather with elem_size_bytes < 256 (non-transpose only).

    bass.dma_gather asserts elem_size_bytes % 256 == 0, but the Q7 ucode
    only needs that for transpose=True; the non-transpose path supports any
    elem_size (one packet per index below 16 KiB). What the ISA does require
    is the table ROW STRIDE in multiples of 256 B (stride_bytes_256 field),
    which the caller satisfies via elem_step. Emitting the instruction here
    keeps the x table 256 B-strided while moving only 128 B per edge.
    """
    import concourse.mybir as mybir

    g = nc.gpsimd
    assert idxs_ap.dtype == mybir.dt.int16
    assert in_ap.dtype == out_ap.dtype
    dt_sz = mybir.dt.size(in_ap.dtype)
    assert in_ap.ap[0][0] == elem_step
    assert in_ap.ap[-1][1] == out_ap.ap[-1][1] == elem_size
    assert num_idxs % 128 == 0
    assert out_ap.ap[0][1] * out_ap.ap[1][1] == num_idxs
    stride_bytes = elem_step * dt_sz
    assert stride_bytes % 256 == 0 and stride_bytes // 256 < 256
    _in_ap = g.lower_ap_dma(in_ap, for_custom_bir_dma=True)
    _idxs_ap = g.lower_ap(idxs_ap)
    _out_ap = g.lower_ap(out_ap)
    return g.add_instruction(
        mybir.InstDMAGatherAnt(
            name=nc.get_next_instruction_name(),
            ins=[*_in_ap, _idxs_ap, g.lower_val_access(g.to_reg(num_idxs))],
            outs=[_out_ap],
            transpose=False,
            num_idxs=num_idxs,
            elem_size=elem_size,
            stride_bytes_256=stride_bytes // 256,
            gen_mode=0,
            single_packet=single_packet,
            queue_num=queue_num,
            sbuf_tokens_per_rank=0,
            sbuf_free_dim_per_rank=0,
            sbuf_free_dim_pad_per_rank=0,
            sbuf_byte_offset=0,
        )
    )


def _build_program(lay_in, lay_out):
    import concourse.bacc as bacc
    import concourse.mybir as mybir
    from concourse import tile
    from concourse import library_config

    f32, f16, i16 = mybir.dt.float32, mybir.dt.float16, mybir.dt.int16
    nc = bacc.Bacc(None, target_bir_lowering=False, debug=False,
                   dynamic_dma_scratch_size=SCRATCH, num_swdge_queues=4)

    xdup = nc.dram_tensor("xdup", [N_NODES, 2 * D], f16, kind="ExternalInput")
    xT_d = nc.dram_tensor("xT", [D, NPC], f32, kind="ExternalInput")
    Wt_d = nc.dram_tensor("Wt", [D, 3 * D], f32, kind="ExternalInput")
    gb_d = nc.dram_tensor("gb", [D, 2], f32, kind="ExternalInput")
    out_d = nc.dram_tensor("out", [D, NPC], f32, kind="ExternalOutput")
    cc_in = nc.dram_tensor("cc_in", [D, 2], f32)
    cc_out = nc.dram_tensor("cc_out", [D, 2], f32, addr_space="Shared")

    lays = {"in": lay_in, "out": lay_out}
    idx_d, dcmp_d = {}, {}
    for dk in ("in", "out"):
        idx_d[dk] = nc.dram_tensor(
            f"idx_{dk}", [128, lays[dk]["total"] // 16], i16,
            kind="ExternalInput")
        dcmp_d[dk] = nc.dram_tensor(
            f"dcmp_{dk}", [128, lays[dk]["n_inst"]], f16,
            kind="ExternalInput")

    with tile.TileContext(nc) as tc:
        nc.gpsimd.load_library(library_config.mlp)
        with (
            tc.tile_pool(name="const", bufs=1) as cpool,
            tc.tile_pool(name="gath", bufs=28) as gpool,
            tc.tile_pool(name="mb", bufs=8) as mpool,
            tc.tile_pool(name="dr", bufs=4) as dpool,
            tc.tile_pool(name="aggt", bufs=6) as apool,
            tc.tile_pool(name="agg_ps", bufs=5, space="PSUM") as agg_pspool,
            tc.tile_pool(name="out_ps", bufs=3, space="PSUM") as out_pspool,
        ):
            # --- constants
            xT = cpool.tile([D, NPC], f32, tag="xT")
            nc.sync.dma_start(xT[:], xT_d[:])
            Wt = cpool.tile([D, 3 * D], f32, tag="Wt")
            nc.sync.dma_start(Wt[:], Wt_d[:])
            gb = cpool.tile([D, 2], f32, tag="gb")
            nc.sync.dma_start(gb[:], gb_d[:])
            iota_i = cpool.tile([128, MB, W], i16, tag="iota_i")
            nc.gpsimd.iota(iota_i[:], [[0, MB], [1, W]], base=0,
                           channel_multiplier=0)
            iota_f = cpool.tile([128, MB, W], f16, tag="iota_f")
            nc.vector.tensor_copy(iota_f[:], iota_i[:])

            idx_t, dcmp_t = {}, {}
            for dk in ("in", "out"):
                idx_t[dk] = cpool.tile(
                    [128, lays[dk]["total"] // 16], i16, tag=f"idx{dk}",
                    name=f"idx_t_{dk}")
                nc.sync.dma_start(idx_t[dk][:], idx_d[dk][:])
                dcmp_t[dk] = cpool.tile(
                    [128, lays[dk]["n_inst"]], f16, tag=f"dc{dk}",
                    name=f"dcmp_t_{dk}")
                nc.sync.dma_start(dcmp_t[dk][:], dcmp_d[dk][:])

            g_tiles = {"in": {}, "out": {}}
            m_tiles = {"in": {}, "out": {}}
            _bcast_failed = []

            g_dummy = None
            if "gather" in ABLATE or "detach" in ABLATE:
                g_dummy = cpool.tile([128, GCAP // 128, D], f16, tag="gdum")
                nc.vector.memset(g_dummy[:], 0.0)

            def emit_run(dk, ri):
                lay = lays[dk]
                p0, npos, h = lay["runs"][ri]
                gt = gpool.tile([128, npos // 128, D], f16, tag="g",
                                name=f"g_{dk}_{ri}")
                src = xdup[h * HALF:(h + 1) * HALF, 0:D]
                qn = (0 if dk == "in" else 2) + h  # one queue per stream
                _dma_gather_narrow(
                    nc, gt[:], src, idx_t[dk][:, p0 // 16:(p0 + npos) // 16],
                    npos, D, 2 * D, queue_num=qn, single_packet=SP)
                g_tiles[dk][ri] = gt

            def emit_batch(dk, k):
                """Build the one-hot masks for instance batch k."""
                nb = min(MB, lays[dk]["n_inst"] - k * MB)
                bcast = dcmp_t[dk][:, k * MB:k * MB + nb].unsqueeze(2) \
                    .broadcast_to([128, nb, W])
                mt = mpool.tile([128, nb, W], f16, tag="m",
                                name=f"m_{dk}_{k}")
                try:
                    nc.vector.tensor_tensor(
                        mt[:], iota_f[:, :nb, :], bcast,
                        op=mybir.AluOpType.is_equal)
                except Exception:
                    _bcast_failed.append(k)
                    dr = dpool.tile([128, nb, W], f16, tag="drep",
                                    name=f"dr_{dk}_{k}")
                    nc.vector.tensor_copy(dr[:], bcast)
                    nc.vector.tensor_tensor(
                        mt[:], iota_f[:, :nb, :], dr[:],
                        op=mybir.AluOpType.is_equal)
                m_tiles[dk][k] = mt

            # --- per-block: accumulate windows, then epilogue
            r_sb = cpool.tile([D, NPC], f32, tag="r")
            sums = cpool.tile([D, NBLK], f32, tag="sums")
            sumsq = cpool.tile([D, NBLK], f32, tag="sumsq")
            sq_scr = cpool.tile([D, 128], f32, tag="sq")

            prefetch = sorted(
                [(lays[dk]["run_first_blk"][ri], dk, ri)
                 for dk in ("in", "out")
                 for ri in range(len(lays[dk]["runs"]))])
            pf_ptr = [0]

            def prefetch_to(blk):
                while (pf_ptr[0] < len(prefetch)
                       and prefetch[pf_ptr[0]][0] <= blk):
                    _, dk, ri = prefetch[pf_ptr[0]]
                    if ri not in g_tiles[dk] and "gather" not in ABLATE:
                        emit_run(dk, ri)
                    pf_ptr[0] += 1

            def emit_epilogue(b, agg_in_t, agg_out_t):
                ncols = min(128, NPC - b * 128)
                ops = out_pspool.tile([D, ncols], f32, tag="outps",
                                      name=f"ops_{b}")
                nc.tensor.matmul(ops[:], Wt[:, 0:D],
                                 xT[:, b * 128:b * 128 + ncols],
                                 start=True, stop=False)
                nc.tensor.matmul(ops[:], Wt[:, D:2 * D],
                                 agg_in_t[:, 0:ncols],
                                 start=False, stop=False)
                nc.tensor.matmul(ops[:], Wt[:, 2 * D:3 * D],
                                 agg_out_t[:, 0:ncols],
                                 start=False, stop=True)
                nc.scalar.activation(
                    r_sb[:, b * 128:b * 128 + ncols], ops[:],
                    mybir.ActivationFunctionType.Relu,
                    accum_out=sums[:, b:b + 1])
                nc.scalar.activation(
                    sq_scr[:, 0:ncols], r_sb[:, b * 128:b * 128 + ncols],
                    mybir.ActivationFunctionType.Square,
                    accum_out=sumsq[:, b:b + 1])

            pending = []
            for b in range(NBLK):
                prefetch_to(b + LA)
                agg_t = {}
                for dk in ("in", "out"):
                    lay = lays[dk]
                    at = apool.tile([D, 128], f32, tag="aggt",
                                    name=f"at_{dk}_{b}")
                    covered = 0
                    for (w, ilist) in lay["block_insts"][b]:
                        col0 = (w % 2) * W
                        if not ilist or "masks" in ABLATE:
                            if "masks" in ABLATE and ilist \
                               and "gather" not in ABLATE:
                                for (ii, c) in ilist:
                                    ri = int(lay["chunk_run"][c])
                                    if ri not in g_tiles[dk]:
                                        emit_run(dk, ri)
                            nc.vector.memset(at[:, col0:col0 + W], 0.0)
                            covered += 1
                            continue
                        aps = agg_pspool.tile([D, W], f32, tag="aggps",
                                              name=f"aps_{dk}_{b}_{w}")
                        for j, (ii, c) in enumerate(ilist):
                            k, r = divmod(ii, MB)
                            if k not in m_tiles[dk]:
                                emit_batch(dk, k)
                            ri = int(lay["chunk_run"][c])
                            if "gather" in ABLATE:
                                gt = g_dummy
                            else:
                                if ri not in g_tiles[dk]:
                                    emit_run(dk, ri)
                                gt = g_tiles[dk][ri]
                            if "detach" in ABLATE:
                                gt = g_dummy
                            nc.tensor.matmul(
                                aps[:],
                                gt[:, (c - int(lay["run_c0"][ri]))
                                   % (GCAP // 128), 0:D],
                                m_tiles[dk][k][:, r, :],
                                start=(j == 0), stop=(j == len(ilist) - 1))
                        nc.scalar.activation(at[:, col0:col0 + W], aps[:],
                                             mybir.ActivationFunctionType.Copy)
                        covered += 1
                    if covered < 2:  # trailing window beyond NPC
                        nc.vector.memset(at[:, W:128], 0.0)
                    agg_t[dk] = at

                # one-block software pipeline: PE starts block b+1's scatter
                # matmuls while Act drains block b's window copies + epilogue
                pending.append((b, agg_t["in"], agg_t["out"]))
                if len(pending) > 1:
                    emit_epilogue(*pending.pop(0))
            for args in pending:
                emit_epilogue(*args)

            # --- BN stats allreduce
            part = cpool.tile([D, 2], f32, tag="part")
            nc.vector.tensor_reduce(part[:, 0:1], sums[:],
                                    mybir.AxisListType.X, mybir.AluOpType.add)
            nc.vector.tensor_reduce(part[:, 1:2], sumsq[:],
                                    mybir.AxisListType.X, mybir.AluOpType.add)
            nc.sync.dma_start(cc_in[:], part[:])
            if "nocc" not in ABLATE:
                nc.gpsimd.collective_compute(
                    "AllReduce", mybir.AluOpType.add,
                    replica_groups=[list(range(N_CORES))],
                    ins=[cc_in[:]], outs=[cc_out[:]])
            tot = cpool.tile([D, 2], f32, tag="tot")
            nc.sync.dma_start(tot[:], cc_out[:] if "nocc" not in ABLATE
                              else cc_in[:])

            # --- scale/shift
            stats = cpool.tile([D, 8], f32, tag="stats")
            mean, ex2 = stats[:, 0:1], stats[:, 1:2]
            var, std = stats[:, 2:3], stats[:, 3:4]
            inv, scale = stats[:, 4:5], stats[:, 5:6]
            shift, tmp = stats[:, 6:7], stats[:, 7:8]
            inv_n = 1.0 / float(N_NODES)
            nc.vector.tensor_scalar_mul(mean, tot[:, 0:1], inv_n)
            nc.vector.tensor_scalar_mul(ex2, tot[:, 1:2], inv_n)
            nc.vector.tensor_tensor(tmp, mean, mean, op=mybir.AluOpType.mult)
            nc.vector.tensor_tensor(var, ex2, tmp,
                                    op=mybir.AluOpType.subtract)
            nc.vector.tensor_scalar_add(var, var, BN_EPS)
            nc.scalar.activation(std, var, mybir.ActivationFunctionType.Sqrt)
            nc.vector.reciprocal(inv, std)
            nc.vector.tensor_tensor(scale, gb[:, 0:1], inv,
                                    op=mybir.AluOpType.mult)
            nc.vector.tensor_tensor(tmp, mean, scale,
                                    op=mybir.AluOpType.mult)
            nc.vector.tensor_tensor(shift, gb[:, 1:2], tmp,
                                    op=mybir.AluOpType.subtract)

            # --- normalize into the retired xT buffer + writeback
            nc.vector.tensor_scalar(xT[:], r_sb[:], scale, shift,
                                    op0=mybir.AluOpType.mult,
                                    op1=mybir.AluOpType.add)
            nc.sync.dma_start(out_d[:], xT[:])

    if _bcast_failed:
        print(f"note: mask broadcast tensor_tensor fell back to copy "
              f"for {len(_bcast_failed)} batches", file=sys.stderr)
    nc.finalize()
    return nc


# ---------------------------------------------------------------- kernel

def prepare(x, edge_index, num_nodes=None, W_in=None, W_out=None,
            W_self=None, gamma=None, beta=None):
    """Build the bass program and per-core input maps."""
    x = np.asarray(x, np.float32)
    edge_index = np.asarray(edge_index, np.int64)
    W_in = np.asarray(W_in, np.float32)
    W_out = np.asarray(W_out, np.float32)
    W_self = np.asarray(W_self, np.float32)
    gamma = np.asarray(gamma, np.float32)
    beta = np.asarray(beta, np.float32)
    assert x.shape == (N_NODES, D) and edge_index.shape == (2, N_EDGES)

    src, dst = edge_index[0], edge_index[1]
    lay_in, pc_in = _route_direction(dst, src)    # agg_in: reduce by dst
    lay_out, pc_out = _route_direction(src, dst)  # agg_out: reduce by src

    nc = _build_program(lay_in, lay_out)

    x16 = x.astype(np.float16)
    xdup = np.concatenate([x16, x16], axis=1)
    Wt = np.concatenate([W_self.T, W_in.T, W_out.T], axis=1).astype(np.float32)
    Wt = np.ascontiguousarray(Wt)
    gb = np.ascontiguousarray(np.stack([gamma, beta], axis=1).astype(np.float32))

    in_maps = []
    for c in range(N_CORES):
        xT_c = np.ascontiguousarray(x[c * NPC:(c + 1) * NPC].T)
        in_maps.append({
            "xdup": xdup,
            "xT": xT_c,
            "Wt": Wt,
            "gb": gb,
            "idx_in": pc_in[c][0], "dcmp_in": pc_in[c][1],
            "idx_out": pc_out[c][0], "dcmp_out": pc_out[c][1],
        })
    return nc, in_maps


def postprocess(results):
    outT = np.concatenate([r["out"] for r in results], axis=1)
    return np.ascontiguousarray(outT.T).astype(np.float32)


def kernel(x, edge_index, num_nodes=None, W_in=None, W_out=None,
           W_self=None, gamma=None, beta=None):
    from concourse.bass_utils import run_bass_kernel_spmd

    nc, in_maps = prepare(x, edge_index, num_nodes, W_in, W_out,
                          W_self, gamma, beta)
    res = run_bass_kernel_spmd(nc, in_maps, core_ids=list(range(N_CORES)))
    return postprocess(res.results)
